# revision 3
# baseline (speedup 1.0000x reference)
"""Trainium2 Bass kernel for masked dot-product attention.

Problem: B=16, Lq=Lk=2048, d=128, fp32.
  scores = Q @ K^T / sqrt(d); mask key positions >= valid_len with -1e6;
  attn = softmax(scores, axis=-1); out = attn @ V.

Strategy
--------
Work is sharded over (batch, half-L): 32 shards of 1024 query rows, grouped
into 4 SPMD slots of 8 cores. Shards are sorted by key-tile count
(ceil(valid_len/128)) and slot s runs ranks [8s, 8s+8); the compiled program
bakes the per-slot key extent E_s = band max, so device work scales with the
actual valid lengths. Each slot DMAs one concatenated bf16 input
(Q^T | K^T | V-tiles | 0/1 mask columns) and runs TWO sequential 512-query
pipeline passes over the same K/V tiles, so K/V are uploaded and DMA'd once
per 1024 queries.

Device pipeline per pass (k-on-partitions layout, all operands bf16, fp32
PSUM accumulation):
  MM1:  S^T[k,q] = (K^T tile).T @ Q^T          (PE, N=512, per k-tile)
  exp:  E = exp(S^T / sqrt(d))                 (ACT, PSUM->SBUF bf16,
                                                GSZ=2 k-tiles per call)
  mask: E rows with k >= valid_len *= 0        (DVE tensor_scalar, only for
                                                tiles that can be partial
                                                in this band)
  MM2:  num^T[d,q] += V_tile.T-layout @ E      (PE accumulate over k-tiles)
  den:  pd += ones^T @ (E_t + E_{t+1})         (one DVE add + one PE matmul
                                                per k-tile group)
MM1/exp of group g run one group ahead of MM2/den of g-1 (software pipeline,
double-buffered PSUM) so PE, ACT and DVE overlap. Masking is exact and fully
on-device (no -1e6 arithmetic: masked rows of E are exactly zero, so num and
den are exact sums over valid keys; softmax needs no max-subtraction since
scores ~ N(0,1) and exp cannot overflow).

Outputs are num^T in bf16 (halves download bytes; ~0.4% rounding, well under
tolerance) and den in f32; the host computes out = (num/den).T per shard.
"""

import math

import numpy as np

B, L, D = 16, 2048, 128
NCORES = 8
QCHUNK = 512
HCHUNK = 1024
NHALF = L // HCHUNK  # 2
NSLOTS = B * NHALF // NCORES  # 4
GSZ = 2
SCALE = 1.0 / math.sqrt(D)

_programs = {}

_TRACE = False
_REPEAT = 1
_last_results = None


def _build_program(extents, mins, repeat=1):
    import concourse.tile as tile
    from concourse import bacc, mybir

    F32 = mybir.dt.float32
    BF16 = mybir.dt.bfloat16
    Tmax = max(extents)

    nc = bacc.Bacc("TRN2")

    ins = {}
    outs = {}
    for s, T in enumerate(extents):
        nmask = T - max(0, mins[s] - 1)
        W = HCHUNK + 2 * T * 128 + nmask
        ins[f"in{s}"] = nc.dram_tensor(f"in{s}", [128, W], BF16, kind="ExternalInput")
        for qc in range(2):
            o = 2 * s + qc
            outs[f"num{o}"] = nc.dram_tensor(f"num{o}", [128, QCHUNK], BF16, kind="ExternalOutput")
            outs[f"den{o}"] = nc.dram_tensor(f"den{o}", [1, QCHUNK], F32, kind="ExternalOutput")

    with tile.TileContext(nc) as tc:
        with (
            tc.tile_pool(name="const", bufs=1) as const,
            tc.tile_pool(name="inp", bufs=2) as inp,
            tc.tile_pool(name="epool", bufs=3) as epool,
            tc.tile_pool(name="gpool", bufs=3) as gpool,
            tc.tile_pool(name="opool", bufs=3) as opool,
            tc.tile_pool(name="dpool", bufs=3) as dpool,
            tc.tile_pool(name="ps_s", bufs=2, space="PSUM") as ps_s,
            tc.tile_pool(name="ps_o", bufs=2, space="PSUM") as ps_o,
            tc.tile_pool(name="ps_d", bufs=2, space="PSUM") as ps_d,
        ):
            ones = const.tile([128, 128], BF16, tag="ones")
            nc.vector.memset(ones, 1.0)
            Wmax = HCHUNK + 2 * Tmax * 128 + Tmax
            for s, T in [(s, T) for _ in range(repeat) for s, T in enumerate(extents)]:
                mn = mins[s]
                nmask = T - max(0, mn - 1)
                W = HCHUNK + 2 * T * 128 + nmask
                it = inp.tile([128, Wmax], BF16, tag="it")
                nc.sync.dma_start(out=it[:, :W], in_=ins[f"in{s}"][:, :])
                kt = it[:, HCHUNK : HCHUNK + T * 128]
                vt = it[:, HCHUNK + T * 128 : HCHUNK + 2 * T * 128]
                if nmask:
                    zc = dpool.tile([128, nmask], F32, tag="zc", name=f"zc{s}")
                    nc.vector.tensor_copy(zc, it[:, W - nmask : W])

                ngroups = (T + GSZ - 1) // GSZ
                for qc in range(2):
                    qt = it[:, qc * QCHUNK : (qc + 1) * QCHUNK]
                    po = ps_o.tile([128, QCHUNK], F32, tag="po", name=f"po{s}_{qc}")
                    pd = ps_d.tile([128, QCHUNK], F32, tag="pd", name=f"pd{s}_{qc}")
                    pending = None
                    for g in range(ngroups + 1):
                        if g < ngroups:
                            gtiles = list(range(g * GSZ, min(g * GSZ + GSZ, T)))
                            gn = len(gtiles)
                            pss = ps_s.tile([128, GSZ * QCHUNK], F32, tag="ps")
                            for j, t in enumerate(gtiles):
                                nc.tensor.matmul(
                                    pss[:, j * QCHUNK : (j + 1) * QCHUNK],
                                    kt[:, t * 128 : (t + 1) * 128],
                                    qt,
                                    start=True,
                                    stop=True,
                                )
                            eg = epool.tile([128, GSZ * QCHUNK], BF16, tag="eg")
                            nc.scalar.activation(
                                eg[:, : gn * QCHUNK],
                                pss[:, : gn * QCHUNK],
                                mybir.ActivationFunctionType.Exp,
                                scale=SCALE,
                            )
                            for j, t in enumerate(gtiles):
                                if t >= mn - 1:
                                    nc.vector.tensor_scalar_mul(
                                        eg[:, j * QCHUNK : (j + 1) * QCHUNK],
                                        eg[:, j * QCHUNK : (j + 1) * QCHUNK],
                                        zc[:, t - (mn - 1) : t - (mn - 1) + 1],
                                    )
                            cur = (g, gtiles, eg)
                        else:
                            cur = None
                        if pending is not None:
                            pg, ptiles, peg = pending
                            for j, t in enumerate(ptiles):
                                es = peg[:, j * QCHUNK : (j + 1) * QCHUNK]
                                nc.tensor.matmul(
                                    po,
                                    vt[:, t * 128 : (t + 1) * 128],
                                    es,
                                    start=(t == 0),
                                    stop=(t == T - 1),
                                )
                            if len(ptiles) == 2:
                                gsum = gpool.tile([128, QCHUNK], BF16, tag="gsum")
                                nc.vector.tensor_add(
                                    gsum, peg[:, 0:QCHUNK], peg[:, QCHUNK:]
                                )
                            else:
                                gsum = peg[:, 0:QCHUNK]
                            nc.tensor.matmul(
                                pd,
                                ones,
                                gsum,
                                start=(pg == 0),
                                stop=(pg == ngroups - 1),
                            )
                        pending = cur
                    o = 2 * s + qc
                    osb = opool.tile([128, QCHUNK], BF16, tag="osb")
                    nc.vector.tensor_copy(osb, po)
                    nc.sync.dma_start(out=outs[f"num{o}"][:, :], in_=osb)
                    dsb = dpool.tile([1, QCHUNK], F32, tag="dsb")
                    nc.vector.tensor_copy(dsb, pd[0:1, :])
                    nc.sync.dma_start(out=outs[f"den{o}"][:, :], in_=dsb)

    nc.finalize()
    return nc


def _get_program_km(extents, mins, repeat=1):
    key = (tuple(extents), tuple(mins), repeat)
    if key not in _programs:
        _programs[key] = _build_program(tuple(extents), tuple(mins), repeat)
    return _programs[key]


def _shard_plan(vl):
    tiles = [max(1, int(math.ceil(int(vl[b]) / 128.0))) for b in range(B)]
    shards = sorted(
        ((tiles[b], b, h) for b in range(B) for h in range(NHALF)),
        key=lambda x: (-x[0], x[1], x[2]),
    )
    extents = tuple(shards[s * NCORES][0] for s in range(NSLOTS))
    mins = tuple(shards[s * NCORES + NCORES - 1][0] for s in range(NSLOTS))
    return shards, extents, mins


def _make_in_maps(queries, keys, values, vl, shards, extents, mins):
    import ml_dtypes

    BF = ml_dtypes.bfloat16
    kcache = {}

    def kv(b, T):
        key = (b, T)
        if key not in kcache:
            n = int(vl[b])
            kt = keys[b, : T * 128].T.astype(BF)
            vt = (
                values[b, : T * 128]
                .reshape(T, 128, D)
                .transpose(1, 0, 2)
                .reshape(128, T * D)
                .astype(BF)
            )
            z = np.zeros((T * 128,), np.float32)
            z[:n] = 1.0
            z = np.ascontiguousarray(z.reshape(T, 128).T).astype(BF)
            kcache[key] = (kt, vt, z)
        return kcache[key]

    qtr = {}

    def qtb(b):
        if b not in qtr:
            qtr[b] = queries[b].T.astype(BF)
        return qtr[b]

    in_maps = [{} for _ in range(NCORES)]
    for s in range(NSLOTS):
        T = extents[s]
        nmask = T - max(0, mins[s] - 1)
        for c in range(NCORES):
            _, b, h = shards[s * NCORES + c]
            kt, vt, z = kv(b, T)
            qt = qtb(b)[:, h * HCHUNK : (h + 1) * HCHUNK]
            in_maps[c][f"in{s}"] = np.concatenate([qt, kt, vt, z[:, T - nmask :]], axis=1)
    return in_maps


def kernel(queries, keys, values, valid_lens):
    from concourse.bass_utils import run_bass_kernel_spmd

    queries = np.ascontiguousarray(queries, dtype=np.float32)
    keys = np.ascontiguousarray(keys, dtype=np.float32)
    values = np.ascontiguousarray(values, dtype=np.float32)
    vl = np.asarray(valid_lens).astype(np.int64).clip(1, L)
    assert queries.shape == (B, L, D), queries.shape

    shards, extents, mins = _shard_plan(vl)
    nc = _get_program_km(extents, mins, _REPEAT)
    in_maps = _make_in_maps(queries, keys, values, vl, shards, extents, mins)

    res = run_bass_kernel_spmd(nc, in_maps, core_ids=list(range(NCORES)), trace=_TRACE)
    globals()["_last_results"] = res

    out = np.empty((B, L, D), np.float32)
    for s in range(NSLOTS):
        for c in range(NCORES):
            _, b, h = shards[s * NCORES + c]
            r = res.results[c]
            for qc in range(2):
                o = 2 * s + qc
                num = r[f"num{o}"].astype(np.float32)
                den = r[f"den{o}"]
                lo = h * HCHUNK + qc * QCHUNK
                out[b, lo : lo + QCHUNK] = (num / den).T
    return out


# revision 4
# speedup vs baseline: 2.1995x; 2.1995x over previous
"""Trainium2 Bass kernel for masked dot-product attention.

Problem: B=16, Lq=Lk=2048, d=128, fp32.
  scores = Q @ K^T / sqrt(d); mask key positions >= valid_len with -1e6;
  attn = softmax(scores, axis=-1); out = attn @ V.

Strategy
--------
Work is sharded over (batch, half-L): 32 shards of 1024 query rows in 4 SPMD
slots of 8 cores, sorted by key-tile count (ceil(valid_len/128)); slot s runs
ranks [8s, 8s+8) and bakes the band-max key extent E_s, so device work scales
with actual valid lengths. Each slot DMAs one concatenated bf16 input
(Q^T | K^T | V-tiles | 0/1 mask columns) and runs TWO sequential 512-query
pipeline passes over the same K/V tiles, so K/V are uploaded and DMA'd once
per 1024 queries.

Device pipeline per pass (k-on-partitions layout, bf16 operands, fp32 PSUM):
  MM1:  S^T[k,q] = (K^T tile).T @ Q^T       (PE, N=512, per k-tile)
  exp:  E = exp(S^T / sqrt(d))              (ACT, PSUM->SBUF bf16, 2 tiles/call)
  MM2:  num^T[d,q] += V_tile @ E            (PE accumulate; V rows at
                                             k >= valid_len are zeroed on the
                                             host, so num needs no mask and E
                                             feeds MM2 straight from ACT)
  den:  masked copies of partial/invalid E tiles (DVE tensor_scalar with the
        0/1 column), pair-summed on DVE, one ones-stationary matmul per pair
        accumulates the denominator off the critical path.
MM1/exp of group g run one group ahead of MM2/den of g-1 (software pipeline,
double-buffered PSUM) so PE, ACT and DVE overlap. Masking is exact: masked E
rows are exactly zero in den and V rows exactly zero in num; softmax needs no
max-subtraction (scores ~ N(0,1), exp cannot overflow fp32).

Outputs: num^T in bf16 (halves download; ~0.4% rounding, well inside the
tolerance) and den in f32; host computes out = (num/den).T per shard.
"""

import math

import numpy as np

B, L, D = 16, 2048, 128
NCORES = 8
QCHUNK = 512
HCHUNK = 1024
NHALF = L // HCHUNK  # 2
NSLOTS = B * NHALF // NCORES  # 4
GSZ = 2
SCALE = 1.0 / math.sqrt(D)

_programs = {}

_TRACE = False
_REPEAT = 1
_last_results = None


def _build_program(extents, mins, repeat=1):
    import concourse.tile as tile
    from concourse import bacc, mybir

    F32 = mybir.dt.float32
    BF16 = mybir.dt.bfloat16
    Tmax = max(extents)

    nc = bacc.Bacc("TRN2")

    ins = {}
    outs = {}
    for s, T in enumerate(extents):
        nmask = T - max(0, mins[s] - 1)
        W = HCHUNK + 2 * T * 128 + nmask
        ins[f"in{s}"] = nc.dram_tensor(f"in{s}", [128, W], BF16, kind="ExternalInput")
        for qc in range(2):
            o = 2 * s + qc
            outs[f"num{o}"] = nc.dram_tensor(f"num{o}", [128, QCHUNK], BF16, kind="ExternalOutput")
            outs[f"den{o}"] = nc.dram_tensor(f"den{o}", [1, QCHUNK], F32, kind="ExternalOutput")

    with tile.TileContext(nc) as tc:
        with (
            tc.tile_pool(name="const", bufs=1) as const,
            tc.tile_pool(name="inp", bufs=3) as inp,
            tc.tile_pool(name="epool", bufs=4) as epool,
            tc.tile_pool(name="gpool", bufs=4) as gpool,
            tc.tile_pool(name="opool", bufs=3) as opool,
            tc.tile_pool(name="dpool", bufs=3) as dpool,
            tc.tile_pool(name="ps_s", bufs=2, space="PSUM") as ps_s,
            tc.tile_pool(name="ps_o", bufs=2, space="PSUM") as ps_o,
            tc.tile_pool(name="ps_d", bufs=2, space="PSUM") as ps_d,
        ):
            ones = const.tile([128, 128], BF16, tag="ones")
            nc.vector.memset(ones, 1.0)
            Wmax = HCHUNK + 2 * Tmax * 128 + Tmax
            for s, T in [(s, T) for _ in range(repeat) for s, T in enumerate(extents)]:
                mn = mins[s]
                nmask = T - max(0, mn - 1)
                W = HCHUNK + 2 * T * 128 + nmask
                it = inp.tile([128, Wmax], BF16, tag="it")
                nc.sync.dma_start(out=it[:, :W], in_=ins[f"in{s}"][:, :])
                kt = it[:, HCHUNK : HCHUNK + T * 128]
                vt = it[:, HCHUNK + T * 128 : HCHUNK + 2 * T * 128]
                if nmask:
                    zc = dpool.tile([128, nmask], F32, tag="zc", name=f"zc{s}")
                    nc.vector.tensor_copy(zc, it[:, W - nmask : W])

                ngroups = (T + GSZ - 1) // GSZ
                for qc in range(2):
                    qt = it[:, qc * QCHUNK : (qc + 1) * QCHUNK]
                    po = ps_o.tile([128, QCHUNK], F32, tag="po", name=f"po{s}_{qc}")
                    pd = ps_d.tile([128, QCHUNK], F32, tag="pd", name=f"pd{s}_{qc}")
                    pending = None
                    for g in range(ngroups + 1):
                        if g < ngroups:
                            gtiles = list(range(g * GSZ, min(g * GSZ + GSZ, T)))
                            gn = len(gtiles)
                            pss = ps_s.tile([128, GSZ * QCHUNK], F32, tag="ps")
                            for j, t in enumerate(gtiles):
                                nc.tensor.matmul(
                                    pss[:, j * QCHUNK : (j + 1) * QCHUNK],
                                    kt[:, t * 128 : (t + 1) * 128],
                                    qt,
                                    start=True,
                                    stop=True,
                                )
                            eg = epool.tile([128, GSZ * QCHUNK], BF16, tag="eg")
                            nc.scalar.activation(
                                eg[:, : gn * QCHUNK],
                                pss[:, : gn * QCHUNK],
                                mybir.ActivationFunctionType.Exp,
                                scale=SCALE,
                            )
                            em = epool.tile(
                                [128, GSZ * QCHUNK], BF16, tag="em", name=f"em{s}_{qc}_{g}"
                            )
                            for j, t in enumerate(gtiles):
                                if t >= mn - 1:
                                    nc.vector.tensor_scalar_mul(
                                        em[:, j * QCHUNK : (j + 1) * QCHUNK],
                                        eg[:, j * QCHUNK : (j + 1) * QCHUNK],
                                        zc[:, t - (mn - 1) : t - (mn - 1) + 1],
                                    )
                            cur = (g, gtiles, eg, em)
                        else:
                            cur = None
                        if pending is not None:
                            pg, ptiles, peg, pem = pending
                            for j, t in enumerate(ptiles):
                                es = peg[:, j * QCHUNK : (j + 1) * QCHUNK]
                                nc.tensor.matmul(
                                    po,
                                    vt[:, t * 128 : (t + 1) * 128],
                                    es,
                                    start=(t == 0),
                                    stop=(t == T - 1),
                                )
                            def _dsrc(j, t):
                                buf = pem if t >= mn - 1 else peg
                                return buf[:, j * QCHUNK : (j + 1) * QCHUNK]

                            if len(ptiles) == 2:
                                gsum = gpool.tile([128, QCHUNK], BF16, tag="gsum")
                                nc.vector.tensor_add(
                                    gsum, _dsrc(0, ptiles[0]), _dsrc(1, ptiles[1])
                                )
                            else:
                                gsum = _dsrc(0, ptiles[0])
                            nc.tensor.matmul(
                                pd,
                                ones,
                                gsum,
                                start=(pg == 0),
                                stop=(pg == ngroups - 1),
                            )
                        pending = cur
                    o = 2 * s + qc
                    osb = opool.tile([128, QCHUNK], BF16, tag="osb")
                    nc.vector.tensor_copy(osb, po)
                    nc.sync.dma_start(out=outs[f"num{o}"][:, :], in_=osb)
                    dsb = dpool.tile([1, QCHUNK], F32, tag="dsb")
                    nc.vector.tensor_copy(dsb, pd[0:1, :])
                    nc.sync.dma_start(out=outs[f"den{o}"][:, :], in_=dsb)

    nc.finalize()
    return nc


def _get_program_km(extents, mins, repeat=1):
    key = (tuple(extents), tuple(mins), repeat)
    if key not in _programs:
        _programs[key] = _build_program(tuple(extents), tuple(mins), repeat)
    return _programs[key]


def _shard_plan(vl):
    tiles = [max(1, int(math.ceil(int(vl[b]) / 128.0))) for b in range(B)]
    shards = sorted(
        ((tiles[b], b, h) for b in range(B) for h in range(NHALF)),
        key=lambda x: (-x[0], x[1], x[2]),
    )
    extents = tuple(shards[s * NCORES][0] for s in range(NSLOTS))
    mins = tuple(shards[s * NCORES + NCORES - 1][0] for s in range(NSLOTS))
    return shards, extents, mins


def _make_in_maps(queries, keys, values, vl, shards, extents, mins):
    import ml_dtypes

    BF = ml_dtypes.bfloat16
    kcache = {}

    def kv(b, T):
        key = (b, T)
        if key not in kcache:
            n = int(vl[b])
            kt = keys[b, : T * 128].T.astype(BF)
            vz = values[b, : T * 128].copy()
            vz[n:] = 0.0
            vt = (
                vz.reshape(T, 128, D)
                .transpose(1, 0, 2)
                .reshape(128, T * D)
                .astype(BF)
            )
            z = np.zeros((T * 128,), np.float32)
            z[:n] = 1.0
            z = np.ascontiguousarray(z.reshape(T, 128).T).astype(BF)
            kcache[key] = (kt, vt, z)
        return kcache[key]

    qtr = {}

    def qtb(b):
        if b not in qtr:
            qtr[b] = queries[b].T.astype(BF)
        return qtr[b]

    in_maps = [{} for _ in range(NCORES)]
    for s in range(NSLOTS):
        T = extents[s]
        nmask = T - max(0, mins[s] - 1)
        for c in range(NCORES):
            _, b, h = shards[s * NCORES + c]
            kt, vt, z = kv(b, T)
            qt = qtb(b)[:, h * HCHUNK : (h + 1) * HCHUNK]
            in_maps[c][f"in{s}"] = np.concatenate([qt, kt, vt, z[:, T - nmask :]], axis=1)
    return in_maps


def kernel(queries, keys, values, valid_lens):
    from concourse.bass_utils import run_bass_kernel_spmd

    queries = np.ascontiguousarray(queries, dtype=np.float32)
    keys = np.ascontiguousarray(keys, dtype=np.float32)
    values = np.ascontiguousarray(values, dtype=np.float32)
    vl = np.asarray(valid_lens).astype(np.int64).clip(1, L)
    assert queries.shape == (B, L, D), queries.shape

    shards, extents, mins = _shard_plan(vl)
    nc = _get_program_km(extents, mins, _REPEAT)
    in_maps = _make_in_maps(queries, keys, values, vl, shards, extents, mins)

    res = run_bass_kernel_spmd(nc, in_maps, core_ids=list(range(NCORES)), trace=_TRACE)
    globals()["_last_results"] = res

    out = np.empty((B, L, D), np.float32)
    for s in range(NSLOTS):
        for c in range(NCORES):
            _, b, h = shards[s * NCORES + c]
            r = res.results[c]
            for qc in range(2):
                o = 2 * s + qc
                num = r[f"num{o}"].astype(np.float32)
                den = r[f"den{o}"]
                lo = h * HCHUNK + qc * QCHUNK
                out[b, lo : lo + QCHUNK] = (num / den).T
    return out
